# revision 3
# baseline (speedup 1.0000x reference)
"""GNN message-passing (3x GraphConv + mean-pool + FC + softmax, graph 0 only)
on 8 Trainium2 NeuronCores — one-hot scatter + bf16 pair-table variant.

Strategy
--------
Nodes are partitioned interleaved across the 8 cores. ``segment_sum(h[src]) @
W_rel`` is computed as ``segment_sum(z[src])`` with ``z = h @ W_rel``; z
tables are stored in **bf16** as a "pair table" of 256-byte rows holding two
consecutive node vectors, which halves the AllGather traffic and keeps the
dma_gather element at the 256B minimum. Each layer:

  1. every core computes ``z`` for its own nodes (cast bf16); shards are
     AllGather-ed into a replicated DRAM pair table,
  2. each core dma_gathers the pair-rows of its in-edges in dense 128-edge
     blocks (~1 gather index per edge; per destination chunk the edges are
     split into even-parity and odd-parity block runs so each block reads one
     64-wide half of the gathered columns),
  3. each block is scattered into its destination columns with a PE matmul
     (bf16) against a one-hot selector built on the Vector engine (batched 4
     blocks per instruction), accumulated in PSUM per 128-destination chunk,
  4. the root term is added and ReLU-ed (feature-major), and the next
     layer's z chunk is written back node-major.

Only ``probs[0]`` is returned by the reference, so layers 2/3 are pruned to
the 1-hop/2-hop in-neighborhoods of graph-0 nodes (exact). Mean-pool partial
sums are AllReduce-d; every core redundantly computes the final FC + softmax.
"""

import numpy as np
import ml_dtypes

import concourse.bacc as bacc
import concourse.bass as bass
import concourse.mybir as mybir
import concourse.tile as tile
from concourse._compat import cdiv
from concourse.bass_utils import run_bass_kernel_spmd
from concourse.masks import make_identity

NCORES = 8
F32 = mybir.dt.float32
BF16 = mybir.dt.bfloat16
I16 = mybir.dt.int16
AX = mybir.AluOpType
ACTF = mybir.ActivationFunctionType

GROUP_ROWS = 48   # gather rows (of 128 idx) per dma_gather call


class Plan:
    pass


def build_plan(x, edge_index, batch):
    p = Plan()
    N, F = x.shape
    src = np.asarray(edge_index[0], dtype=np.int64)
    dst = np.asarray(edge_index[1], dtype=np.int64)
    batch = np.asarray(batch, dtype=np.int64)

    NPC = cdiv(N, NCORES)
    NLOC = cdiv(NPC + 1, 128) * 128
    p.N, p.F, p.NPC, p.NLOC = N, F, NPC, NLOC

    # --- pruning sets -------------------------------------------------------
    in_T0 = batch == 0
    p.n0 = int(in_T0.sum())
    e3 = in_T0[dst]
    in_T2 = in_T0.copy()
    in_T2[src[e3]] = True
    e2 = in_T2[dst]

    # --- balanced interleaved node->core assignment -------------------------
    nodes = np.arange(N)
    g0 = nodes[in_T0]
    g1 = nodes[in_T2 & ~in_T0]
    g2 = nodes[~in_T2]
    order = np.concatenate([g0, g1, g2])
    j = np.arange(N)
    node_core = np.empty(N, np.int64)
    pos = np.empty(N, np.int64)
    node_core[order] = j % NCORES
    pos[order] = j // NCORES
    n0_k = np.bincount(j[:len(g0)] % NCORES, minlength=NCORES)
    n2_k = np.bincount(j[:len(g0) + len(g1)] % NCORES, minlength=NCORES)
    core_of = node_core[dst]
    p.pos = pos
    p.node_core = node_core

    # --- chunk counts -------------------------------------------------------
    C1 = NLOC // 128
    C3 = max(1, cdiv(int(n0_k.max()), 128))
    C2 = max(C3, cdiv(int(n2_k.max()) + 1, 128))
    C2 = min(C2, C1)
    p.C = [C1, C2, C3]
    p.Z = [NLOC, NLOC, C2 * 128]          # node rows per core per table
    assert all((z * NCORES) // 2 < 32768 for z in p.Z)

    # --- per-layer block layout --------------------------------------------
    # Edges keyed by (dst chunk, parity of table row); per (chunk, parity)
    # the block count is the max over cores. The idx stream per chunk is
    # [even blocks][odd blocks]; groups own whole chunks.
    layers = []
    for li, (emask, Cl) in enumerate([(None, C1), (e2, C2), (e3, C3)]):
        es = src if emask is None else src[emask]
        ed = dst if emask is None else dst[emask]
        ecore = core_of if emask is None else core_of[emask]
        Zl = p.Z[li]
        if li < 2:
            srcpos = node_core[es] * NLOC + pos[es]
        else:
            srcpos = node_core[es] * Zl + pos[es]
        col = pos[ed]
        chunk = col // 128
        lane = col % 128
        par = srcpos % 2
        nblk = np.zeros((Cl, 2), np.int64)
        for k in range(NCORES):
            m = ecore == k
            for h in (0, 1):
                cnt = np.bincount(chunk[m & (par == h)], minlength=Cl)
                nblk[:, h] = np.maximum(nblk[:, h], cdiv(cnt, 128))
        layers.append(dict(li=li, Cl=Cl, Zl=Zl, srcpos=srcpos, col=col,
                           ecore=ecore, chunk=chunk, lane=lane, par=par,
                           nblk=nblk))
    p.layers = layers

    # --- global block/group layout (shared) ---------------------------------
    blk_off = 0
    for L in layers:
        L["blk_even"] = {}
        L["blk_odd"] = {}
        groups = []
        cur, cur_rows = [], 0
        for c in range(L["Cl"]):
            r = int(L["nblk"][c].sum())
            if cur and cur_rows + r > GROUP_ROWS:
                groups.append(cur)
                cur, cur_rows = [], 0
            cur.append(c)
            cur_rows += r
        if cur:
            groups.append(cur)
        # (blk_start, nrows, [(c, [(local_row, parity)] )])
        L["groups"] = []
        for grp in groups:
            blk_start = blk_off
            chunks = []
            for c in grp:
                mms = []
                for h, offmap in ((0, L["blk_even"]), (1, L["blk_odd"])):
                    nb = int(L["nblk"][c, h])
                    offmap[c] = blk_off
                    for b in range(nb):
                        mms.append((blk_off - blk_start + b, h))
                    blk_off += nb
                chunks.append((c, mms))
            nrows = blk_off - blk_start
            if nrows:
                L["groups"].append((blk_start, nrows, chunks))
    p.total_blocks = blk_off
    p.WTOT = blk_off * 8

    # --- per-core idx + dstcol arrays ---------------------------------------
    pad_pair = [(NLOC - 2) // 2, (NLOC - 2) // 2, (p.Z[2] - 2) // 2]
    p.idx = []
    p.dstcol = []
    for k in range(NCORES):
        flat = np.zeros(p.total_blocks * 128, np.int16)
        dcol = np.full(p.total_blocks * 128, 999.0, np.float32)
        for L in layers:
            li = L["li"]
            m = L["ecore"] == k
            sp_ = L["srcpos"][m]
            ch_ = L["chunk"][m]
            ln_ = L["lane"][m]
            pr_ = L["par"][m]
            for h, offmap in ((0, L["blk_even"]), (1, L["blk_odd"])):
                for c, off in offmap.items():
                    nb = int(L["nblk"][c, h])
                    if nb:
                        flat[off * 128:(off + nb) * 128] = pad_pair[li]
            order2 = np.lexsort((sp_, pr_, ch_))
            sc = ch_[order2]
            sh = pr_[order2]
            spv = sp_[order2]
            lnv = ln_[order2]
            key = sh.astype(np.int64) * (1 << 32) + sc
            newgrp = np.r_[True, np.diff(key) != 0]
            starts = np.flatnonzero(newgrp)
            lens = np.diff(np.r_[starts, len(key)])
            rank = np.arange(len(key)) - np.repeat(starts, lens)
            base_e = np.array([L["blk_even"].get(c, 0)
                               for c in range(L["Cl"])])
            base_o = np.array([L["blk_odd"].get(c, 0)
                               for c in range(L["Cl"])])
            base = np.where(sh == 0, base_e[sc], base_o[sc])
            slot = base * 128 + rank
            flat[slot] = (spv // 2).astype(np.int16)
            dcol[slot] = lnv.astype(np.float32)
        wrapped = flat.reshape(-1, 16).T.copy()
        p.idx.append(np.tile(wrapped, (8, 1)))
        p.dstcol.append(np.ascontiguousarray(
            dcol.reshape(p.total_blocks, 128).T.astype(ml_dtypes.bfloat16)))

    # --- per-core xT --------------------------------------------------------
    p.xT = []
    for k in range(NCORES):
        xp = np.zeros((NLOC, F), np.float32)
        kn = nodes[node_core == k]
        xp[pos[kn]] = x[kn]
        p.xT.append(np.ascontiguousarray(xp.T))

    # --- per-core pool mask -------------------------------------------------
    p.mask = []
    for k in range(NCORES):
        msk = np.zeros(C3 * 128, np.float32)
        msk[:n0_k[k]] = 1.0
        p.mask.append(np.broadcast_to(msk, (64, C3 * 128)).copy())

    return p


# ----------------------------------------------------------------------------
# Device program
# ----------------------------------------------------------------------------

def build_program(p, W, skip_collectives=False, repeat=1):
    nc = bacc.Bacc("TRN2")
    NLOC, F = p.NLOC, p.F
    C1, C2, C3 = p.C
    CL = [C1, C2, C3]

    xT_d = nc.dram_tensor("xT", [F, NLOC], F32, kind="ExternalInput")
    idx_d = nc.dram_tensor("idx", [128, p.WTOT], I16, kind="ExternalInput")
    dst_d = nc.dram_tensor("dstcol", [128, p.total_blocks], BF16,
                           kind="ExternalInput")
    mask_d = nc.dram_tensor("mask", [64, C3 * 128], F32, kind="ExternalInput")
    wr1_d = nc.dram_tensor("W_rel1", [F, 64], F32, kind="ExternalInput")
    wo1_d = nc.dram_tensor("W_root1", [F, 64], F32, kind="ExternalInput")
    wr2_d = nc.dram_tensor("W_rel2", [64, 64], F32, kind="ExternalInput")
    wo2_d = nc.dram_tensor("W_root2", [64, 64], F32, kind="ExternalInput")
    wr3_d = nc.dram_tensor("W_rel3", [64, 64], F32, kind="ExternalInput")
    wo3_d = nc.dram_tensor("W_root3", [64, 64], F32, kind="ExternalInput")
    wfc_d = nc.dram_tensor("W_fc", [64, 10], F32, kind="ExternalInput")
    bfc_d = nc.dram_tensor("b_fc", [1, 10], F32, kind="ExternalInput")
    iota_d = nc.dram_tensor("iota", [128, 128], BF16, kind="ExternalInput")
    out_d = nc.dram_tensor("probs", [1, 10], F32, kind="ExternalOutput")

    rg = [list(range(NCORES))]

    with tile.TileContext(nc) as tc:
        with (
            tc.tile_pool(name="const", bufs=1) as cpool,
            tc.tile_pool(name="persist", bufs=1) as ppool,
            tc.tile_pool(name="stream", bufs=3) as spool,
            tc.tile_pool(name="sel", bufs=6) as selpool,
            tc.tile_pool(name="gather", bufs=5) as gpool,
            tc.tile_pool(name="psum", bufs=1, space="PSUM") as psum,
            tc.tile_pool(name="dram", bufs=1, space="DRAM") as dram,
        ):
            ident = cpool.tile([128, 128], F32)
            make_identity(nc, ident[:])
            wr1_s = cpool.tile([F, 64], F32, tag="wr1")
            wo1_s = cpool.tile([F, 64], F32, tag="wo1")
            w64 = {}
            for nm, d in [("wr2", wr2_d), ("wo2", wo2_d),
                          ("wr3", wr3_d), ("wo3", wo3_d)]:
                w64[nm] = cpool.tile([64, 64], F32, tag=nm, name=nm)
                nc.sync.dma_start(w64[nm][:], d[:])
            nc.sync.dma_start(wr1_s[:], wr1_d[:])
            nc.sync.dma_start(wo1_s[:], wo1_d[:])
            wfc_s = cpool.tile([64, 10], F32, tag="wfc")
            nc.sync.dma_start(wfc_s[:], wfc_d[:])
            bfc_s = cpool.tile([1, 10], F32, tag="bfc")
            nc.sync.dma_start(bfc_s[:], bfc_d[:])
            mask_s = cpool.tile([64, C3 * 128], F32, tag="mask")
            nc.sync.dma_start(mask_s[:], mask_d[:])
            idx_s = cpool.tile([128, p.WTOT], I16, tag="idx")
            nc.sync.dma_start(idx_s[:], idx_d[:])
            dst_s = cpool.tile([128, p.total_blocks], BF16, tag="dst")
            nc.sync.dma_start(dst_s[:], dst_d[:])
            xTall = cpool.tile([F, NLOC], F32, tag="xTall")
            nc.sync.dma_start(xTall[:], xT_d[:])
            zrow = cpool.tile([1, 128], BF16, tag="zrow")
            nc.vector.memset(zrow[:], 0.0)
            iota_s = cpool.tile([128, 128], BF16, tag="iota")
            nc.sync.dma_start(iota_s[:], iota_d[:])

            hT = [ppool.tile([64, NLOC], F32, tag="h1T", name="h1T"),
                  ppool.tile([64, C2 * 128], F32, tag="h2T", name="h2T"),
                  ppool.tile([64, C3 * 128], F32, tag="h3T", name="h3T")]
            rT = [ppool.tile([64, NLOC], F32, tag="r1T", name="r1T"),
                  ppool.tile([64, C2 * 128], F32, tag="r2T", name="r2T"),
                  ppool.tile([64, C3 * 128], F32, tag="r3T", name="r3T")]

            def store_z_chunks(li, zsrcT, c):
                """z chunk c -> bf16 node-major rows of the pair table."""
                wrel = wr1_s if li == 0 else w64["wr%d" % (li + 1)]
                sl = slice(c * 128, (c + 1) * 128)
                znT_p = psum.tile([64, 128], F32, tag="znT", bufs=2)
                nc.tensor.matmul(znT_p[:], lhsT=wrel[:], rhs=zsrcT[:, sl],
                                 start=True, stop=True)
                znT_s = spool.tile([64, 128], F32, tag="znT_s")
                nc.scalar.activation(znT_s[:], znT_p[:], ACTF.Copy)
                zn_p = psum.tile([128, 64], F32, tag="zn", bufs=2)
                nc.tensor.transpose(zn_p[:], znT_s[:], ident[:64, :64])
                zn_s = spool.tile([128, 64], BF16, tag="zn_s")
                nc.scalar.activation(zn_s[:], zn_p[:], ACTF.Copy)
                nc.sync.dma_start(z_own[li][c * 64:(c + 1) * 64, :], zn_s[:])

            def kick_ag(li):
                zp = p.Z[li] // 2
                nc.gpsimd.dma_start(z_own[li][zp - 1:zp, :], zrow[:])
                if not skip_collectives:
                    nc.gpsimd.collective_compute(
                        "AllGather", AX.bypass, replica_groups=rg,
                        ins=[z_own[li].opt()], outs=[z_tab[li].opt()])

            def root_precompute(li, srcT):
                wroot = wo1_s if li == 0 else w64["wo%d" % (li + 1)]
                for c in range(CL[li]):
                    sl = slice(c * 128, (c + 1) * 128)
                    rp = psum.tile([64, 128], F32, tag="znT", bufs=2)
                    nc.tensor.matmul(rp[:], lhsT=wroot[:], rhs=srcT[:, sl],
                                     start=True, stop=True)
                    nc.scalar.activation(rT[li][:, sl], rp[:], ACTF.Copy)

            for _rep in range(repeat):
                # pair tables: [node_rows/2, 128] bf16
                z_own = [dram.tile([p.Z[i] // 2, 128], BF16,
                                   name="z%do_%d" % (i, _rep))
                         for i in range(3)]
                z_tab = [dram.tile([NCORES * p.Z[i] // 2, 128], BF16,
                                   addr_space="Shared",
                                   name="z%dt_%d" % (i, _rep))
                         for i in range(3)]
                pool_in = dram.tile([64, 1], F32, name="pool_in_%d" % _rep)
                pool_out = dram.tile([64, 1], F32, addr_space="Shared",
                                     name="pool_out_%d" % _rep)
                for c in range(C1):
                    store_z_chunks(0, xTall, c)
                kick_ag(0)
                root_precompute(0, xTall)

                for li in range(3):
                    L = p.layers[li]
                    seen = set()
                    for (blk_start, nrows, chunks) in L["groups"]:
                        g = gpool.tile([128, nrows, 128], BF16, tag="G",
                                       name="G")
                        nc.gpsimd.dma_gather(
                            g[:], z_tab[li][:],
                            idx_s[:, blk_start * 8:(blk_start + nrows) * 8],
                            nrows * 128, nrows * 128, 128,
                            single_packet=False)
                        for (c, mms) in chunks:
                            seen.add(c)
                            ntot = len(mms)
                            if ntot == 0:
                                sl = slice(c * 128, (c + 1) * 128)
                                nc.vector.tensor_scalar_max(
                                    hT[li][:, sl], rT[li][:, sl], 0.0)
                                if li < 2 and c * 128 < p.Z[li + 1]:
                                    store_z_chunks(li + 1, hT[li], c)
                                continue
                            agg_p = psum.tile([64, 128], F32, tag="agg",
                                              bufs=3,
                                              name="agg%d_%d" % (li, c))
                            # batched sel builds over consecutive blocks
                            i = 0
                            b0 = 0
                            while b0 < ntot:
                                bn = min(4, ntot - b0)
                                gb = blk_start + mms[b0][0]
                                sel = selpool.tile([128, bn, 128], BF16,
                                                   tag="sel", name="sel")
                                nc.vector.tensor_tensor(
                                    out=sel[:],
                                    in0=dst_s[:, gb:gb + bn].unsqueeze(2)
                                    .to_broadcast([128, bn, 128]),
                                    in1=iota_s[:].unsqueeze(1)
                                    .to_broadcast([128, bn, 128]),
                                    op=AX.is_equal)
                                for b in range(bn):
                                    row, par = mms[b0 + b]
                                    nc.tensor.matmul(
                                        agg_p[:],
                                        lhsT=g[:, row,
                                               par * 64:(par + 1) * 64],
                                        rhs=sel[:, b, :],
                                        start=(i == 0),
                                        stop=(i == ntot - 1))
                                    i += 1
                                b0 += bn
                            sl = slice(c * 128, (c + 1) * 128)
                            nc.vector.tensor_tensor(
                                out=hT[li][:, sl], in0=agg_p[:],
                                in1=rT[li][:, sl], op=AX.add)
                            nc.vector.tensor_scalar_max(
                                hT[li][:, sl], hT[li][:, sl], 0.0)
                            if li < 2 and c * 128 < p.Z[li + 1]:
                                store_z_chunks(li + 1, hT[li], c)
                    for c in range(L["Cl"]):
                        if c not in seen:
                            sl = slice(c * 128, (c + 1) * 128)
                            nc.vector.tensor_scalar_max(
                                hT[li][:, sl], rT[li][:, sl], 0.0)
                            if li < 2 and c * 128 < p.Z[li + 1]:
                                store_z_chunks(li + 1, hT[li], c)
                    if li < 2:
                        kick_ag(li + 1)
                        root_precompute(li + 1, hT[li])

                hm = spool.tile([64, C3 * 128], F32, tag="hm")
                _tail(nc, tc, spool, psum, hm, hT, mask_s, wfc_s,
                      bfc_s, pool_in, pool_out, out_d, p, rg,
                      skip_collectives)

    nc.compile()
    return nc


def _tail(nc, tc, spool, psum, hm, hT, mask_s, wfc_s, bfc_s, pool_in,
          pool_out, out_d, p, rg, skip_collectives=False):
    nc.vector.tensor_tensor(out=hm[:], in0=hT[2][:],
                            in1=mask_s[:], op=AX.mult)
    psum_pool = spool.tile([64, 1], F32, tag="ppart")
    nc.vector.tensor_reduce(psum_pool[:], hm[:],
                            axis=mybir.AxisListType.X, op=AX.add)
    nc.sync.dma_start(pool_in[:], psum_pool[:])
    if not skip_collectives:
        nc.gpsimd.collective_compute(
            "AllReduce", AX.add, replica_groups=rg,
            ins=[pool_in.opt()], outs=[pool_out.opt()])
    pooled = spool.tile([64, 1], F32, tag="pooled")
    nc.sync.dma_start(pooled[:], pool_out[:])
    mean_s = spool.tile([64, 1], F32, tag="mean")
    nc.vector.tensor_scalar_mul(mean_s[:], pooled[:], 1.0 / max(p.n0, 1))
    lg_p = psum.tile([1, 10], F32, tag="lg")
    nc.tensor.matmul(lg_p[:], lhsT=mean_s[:], rhs=wfc_s[:],
                     start=True, stop=True)
    logits = spool.tile([1, 10], F32, tag="logits")
    nc.vector.tensor_tensor(out=logits[:], in0=lg_p[:],
                            in1=bfc_s[:], op=AX.add)
    mx = spool.tile([1, 1], F32, tag="mx")
    nc.vector.tensor_reduce(mx[:], logits[:],
                            axis=mybir.AxisListType.X, op=AX.max)
    nmx = spool.tile([1, 1], F32, tag="nmx")
    nc.vector.tensor_scalar_mul(nmx[:], mx[:], -1.0)
    es = spool.tile([1, 10], F32, tag="es")
    nc.scalar.activation(es[:], logits[:], ACTF.Exp, bias=nmx[:, 0:1])
    ssum = spool.tile([1, 1], F32, tag="ssum")
    nc.vector.tensor_reduce(ssum[:], es[:],
                            axis=mybir.AxisListType.X, op=AX.add)
    inv = spool.tile([1, 1], F32, tag="inv")
    nc.vector.reciprocal(inv[:], ssum[:])
    probs_s = spool.tile([1, 10], F32, tag="probs")
    nc.vector.tensor_scalar_mul(probs_s[:], es[:], inv[:, 0:1])
    nc.sync.dma_start(out_d[:], probs_s[:])


# ----------------------------------------------------------------------------
# Entry point
# ----------------------------------------------------------------------------

def _prep(inputs):
    x = np.ascontiguousarray(np.asarray(inputs["x"], np.float32))
    edge_index = np.asarray(inputs["edge_index"])
    batch = np.asarray(inputs["batch"])
    W = {k: np.ascontiguousarray(np.asarray(inputs[k], np.float32))
         for k in ["W_rel1", "W_root1", "W_rel2", "W_root2",
                   "W_rel3", "W_root3", "W_fc", "b_fc"]}
    p = build_plan(x, edge_index, batch)
    nc = build_program(p, W)
    return nc, _in_maps(p, W)


def _in_maps(p, W):
    in_maps = []
    for k in range(NCORES):
        in_maps.append({
            "xT": p.xT[k], "idx": p.idx[k], "mask": p.mask[k],
            "dstcol": p.dstcol[k],
            "W_rel1": W["W_rel1"], "W_root1": W["W_root1"],
            "W_rel2": W["W_rel2"], "W_root2": W["W_root2"],
            "W_rel3": W["W_rel3"], "W_root3": W["W_root3"],
            "W_fc": W["W_fc"], "b_fc": W["b_fc"].reshape(1, 10),
            "iota": np.tile(np.arange(128, dtype=np.float32)
                            .astype(ml_dtypes.bfloat16), (128, 1)),
        })
    return in_maps


def kernel(**inputs) -> np.ndarray:
    nc, in_maps = _prep(inputs)
    res = run_bass_kernel_spmd(nc, in_maps, list(range(NCORES)))
    return np.asarray(res.results[0]["probs"]).reshape(10).astype(np.float32)


# revision 4
# speedup vs baseline: 1.0432x; 1.0432x over previous
"""GNN message-passing (3x GraphConv + mean-pool + FC + softmax, graph 0 only)
on 8 Trainium2 NeuronCores — one-hot scatter + bf16 pair-table variant.

Strategy
--------
Nodes are partitioned interleaved across the 8 cores. ``segment_sum(h[src]) @
W_rel`` is computed as ``segment_sum(z[src])`` with ``z = h @ W_rel``; z
tables are stored in **bf16** as a "pair table" of 256-byte rows holding two
consecutive node vectors, which halves the AllGather traffic and keeps the
dma_gather element at the 256B minimum. Each layer:

  1. every core computes ``z`` for its own nodes (cast bf16); shards are
     AllGather-ed into a replicated DRAM pair table,
  2. each core dma_gathers the pair-rows of its in-edges in dense 128-edge
     blocks (~1 gather index per edge; per destination chunk the edges are
     split into even-parity and odd-parity block runs so each block reads one
     64-wide half of the gathered columns),
  3. each block is scattered into its destination columns with a PE matmul
     (bf16) against a one-hot selector built on the Vector engine (batched 4
     blocks per instruction), accumulated in PSUM per 128-destination chunk,
  4. the root term is added and ReLU-ed (feature-major), and the next
     layer's z chunk is written back node-major.

Only ``probs[0]`` is returned by the reference, so layers 2/3 are pruned to
the 1-hop/2-hop in-neighborhoods of graph-0 nodes (exact). Mean-pool partial
sums are AllReduce-d; every core redundantly computes the final FC + softmax.
"""

import numpy as np
import ml_dtypes

import concourse.bacc as bacc
import concourse.bass as bass
import concourse.mybir as mybir
import concourse.tile as tile
from concourse._compat import cdiv
from concourse.bass_utils import run_bass_kernel_spmd
from concourse.masks import make_identity

NCORES = 8
F32 = mybir.dt.float32
BF16 = mybir.dt.bfloat16
I16 = mybir.dt.int16
AX = mybir.AluOpType
ACTF = mybir.ActivationFunctionType

GROUP_ROWS = 64   # gather rows (of 128 idx) per dma_gather call


class Plan:
    pass


def build_plan(x, edge_index, batch):
    p = Plan()
    N, F = x.shape
    src = np.asarray(edge_index[0], dtype=np.int64)
    dst = np.asarray(edge_index[1], dtype=np.int64)
    batch = np.asarray(batch, dtype=np.int64)

    NPC = cdiv(N, NCORES)
    NLOC = cdiv(NPC + 1, 128) * 128
    p.N, p.F, p.NPC, p.NLOC = N, F, NPC, NLOC

    # --- pruning sets -------------------------------------------------------
    in_T0 = batch == 0
    p.n0 = int(in_T0.sum())
    e3 = in_T0[dst]
    in_T2 = in_T0.copy()
    in_T2[src[e3]] = True
    e2 = in_T2[dst]

    # --- balanced interleaved node->core assignment -------------------------
    nodes = np.arange(N)
    g0 = nodes[in_T0]
    g1 = nodes[in_T2 & ~in_T0]
    g2 = nodes[~in_T2]
    order = np.concatenate([g0, g1, g2])
    j = np.arange(N)
    node_core = np.empty(N, np.int64)
    pos = np.empty(N, np.int64)
    node_core[order] = j % NCORES
    pos[order] = j // NCORES
    n0_k = np.bincount(j[:len(g0)] % NCORES, minlength=NCORES)
    n2_k = np.bincount(j[:len(g0) + len(g1)] % NCORES, minlength=NCORES)
    core_of = node_core[dst]
    p.pos = pos
    p.node_core = node_core

    # --- chunk counts -------------------------------------------------------
    C1 = NLOC // 128
    C3 = max(1, cdiv(int(n0_k.max()), 128))
    C2 = max(C3, cdiv(int(n2_k.max()) + 1, 128))
    C2 = min(C2, C1)
    p.C = [C1, C2, C3]
    p.Z = [NLOC, NLOC, C2 * 128]          # node rows per core per table
    assert all((z * NCORES) // 2 < 32768 for z in p.Z)

    # --- per-layer block layout --------------------------------------------
    # Edges keyed by (dst chunk, parity of table row); per (chunk, parity)
    # the block count is the max over cores. The idx stream per chunk is
    # [even blocks][odd blocks]; groups own whole chunks.
    layers = []
    for li, (emask, Cl) in enumerate([(None, C1), (e2, C2), (e3, C3)]):
        es = src if emask is None else src[emask]
        ed = dst if emask is None else dst[emask]
        ecore = core_of if emask is None else core_of[emask]
        Zl = p.Z[li]
        if li < 2:
            srcpos = node_core[es] * NLOC + pos[es]
        else:
            srcpos = node_core[es] * Zl + pos[es]
        col = pos[ed]
        chunk = col // 128
        lane = col % 128
        par = srcpos % 2
        nblk = np.zeros((Cl, 2), np.int64)
        for k in range(NCORES):
            m = ecore == k
            for h in (0, 1):
                cnt = np.bincount(chunk[m & (par == h)], minlength=Cl)
                nblk[:, h] = np.maximum(nblk[:, h], cdiv(cnt, 128))
        layers.append(dict(li=li, Cl=Cl, Zl=Zl, srcpos=srcpos, col=col,
                           ecore=ecore, chunk=chunk, lane=lane, par=par,
                           nblk=nblk))
    p.layers = layers

    # --- global block/group layout (shared) ---------------------------------
    blk_off = 0
    for L in layers:
        L["blk_even"] = {}
        L["blk_odd"] = {}
        groups = []
        cur, cur_rows = [], 0
        for c in range(L["Cl"]):
            r = int(L["nblk"][c].sum())
            if cur and cur_rows + r > GROUP_ROWS:
                groups.append(cur)
                cur, cur_rows = [], 0
            cur.append(c)
            cur_rows += r
        if cur:
            groups.append(cur)
        # (blk_start, nrows, [(c, [(local_row, parity)] )])
        L["groups"] = []
        for grp in groups:
            blk_start = blk_off
            chunks = []
            for c in grp:
                mms = []
                for h, offmap in ((0, L["blk_even"]), (1, L["blk_odd"])):
                    nb = int(L["nblk"][c, h])
                    offmap[c] = blk_off
                    for b in range(nb):
                        mms.append((blk_off - blk_start + b, h))
                    blk_off += nb
                chunks.append((c, mms))
            nrows = blk_off - blk_start
            if nrows:
                L["groups"].append((blk_start, nrows, chunks))
    p.total_blocks = blk_off
    p.WTOT = blk_off * 8

    # --- per-core idx + dstcol arrays ---------------------------------------
    pad_pair = [(NLOC - 2) // 2, (NLOC - 2) // 2, (p.Z[2] - 2) // 2]
    p.idx = []
    p.dstcol = []
    for k in range(NCORES):
        flat = np.zeros(p.total_blocks * 128, np.int16)
        dcol = np.full(p.total_blocks * 128, 999.0, np.float32)
        for L in layers:
            li = L["li"]
            m = L["ecore"] == k
            sp_ = L["srcpos"][m]
            ch_ = L["chunk"][m]
            ln_ = L["lane"][m]
            pr_ = L["par"][m]
            for h, offmap in ((0, L["blk_even"]), (1, L["blk_odd"])):
                for c, off in offmap.items():
                    nb = int(L["nblk"][c, h])
                    if nb:
                        flat[off * 128:(off + nb) * 128] = pad_pair[li]
            order2 = np.lexsort((sp_, pr_, ch_))
            sc = ch_[order2]
            sh = pr_[order2]
            spv = sp_[order2]
            lnv = ln_[order2]
            key = sh.astype(np.int64) * (1 << 32) + sc
            newgrp = np.r_[True, np.diff(key) != 0]
            starts = np.flatnonzero(newgrp)
            lens = np.diff(np.r_[starts, len(key)])
            rank = np.arange(len(key)) - np.repeat(starts, lens)
            base_e = np.array([L["blk_even"].get(c, 0)
                               for c in range(L["Cl"])])
            base_o = np.array([L["blk_odd"].get(c, 0)
                               for c in range(L["Cl"])])
            base = np.where(sh == 0, base_e[sc], base_o[sc])
            slot = base * 128 + rank
            flat[slot] = (spv // 2).astype(np.int16)
            dcol[slot] = lnv.astype(np.float32)
        wrapped = flat.reshape(-1, 16).T.copy()
        p.idx.append(np.tile(wrapped, (8, 1)))
        p.dstcol.append(np.ascontiguousarray(
            dcol.reshape(p.total_blocks, 128).T.astype(ml_dtypes.bfloat16)))

    # --- per-core xT --------------------------------------------------------
    p.xT = []
    for k in range(NCORES):
        xp = np.zeros((NLOC, F), np.float32)
        kn = nodes[node_core == k]
        xp[pos[kn]] = x[kn]
        p.xT.append(np.ascontiguousarray(xp.T))

    # --- per-core pool mask -------------------------------------------------
    p.mask = []
    for k in range(NCORES):
        msk = np.zeros(C3 * 128, np.float32)
        msk[:n0_k[k]] = 1.0
        p.mask.append(np.broadcast_to(msk, (64, C3 * 128)).copy())

    return p


# ----------------------------------------------------------------------------
# Device program
# ----------------------------------------------------------------------------

def build_program(p, W, skip_collectives=False, repeat=1):
    nc = bacc.Bacc("TRN2")
    NLOC, F = p.NLOC, p.F
    C1, C2, C3 = p.C
    CL = [C1, C2, C3]

    xT_d = nc.dram_tensor("xT", [F, NLOC], F32, kind="ExternalInput")
    idx_d = nc.dram_tensor("idx", [128, p.WTOT], I16, kind="ExternalInput")
    dst_d = nc.dram_tensor("dstcol", [128, p.total_blocks], BF16,
                           kind="ExternalInput")
    mask_d = nc.dram_tensor("mask", [64, C3 * 128], F32, kind="ExternalInput")
    wr1_d = nc.dram_tensor("W_rel1", [F, 64], F32, kind="ExternalInput")
    wo1_d = nc.dram_tensor("W_root1", [F, 64], F32, kind="ExternalInput")
    wr2_d = nc.dram_tensor("W_rel2", [64, 64], F32, kind="ExternalInput")
    wo2_d = nc.dram_tensor("W_root2", [64, 64], F32, kind="ExternalInput")
    wr3_d = nc.dram_tensor("W_rel3", [64, 64], F32, kind="ExternalInput")
    wo3_d = nc.dram_tensor("W_root3", [64, 64], F32, kind="ExternalInput")
    wfc_d = nc.dram_tensor("W_fc", [64, 10], F32, kind="ExternalInput")
    bfc_d = nc.dram_tensor("b_fc", [1, 10], F32, kind="ExternalInput")
    iota_d = nc.dram_tensor("iota", [128, 128], BF16, kind="ExternalInput")
    out_d = nc.dram_tensor("probs", [1, 10], F32, kind="ExternalOutput")

    rg = [list(range(NCORES))]

    with tile.TileContext(nc) as tc:
        with (
            tc.tile_pool(name="const", bufs=1) as cpool,
            tc.tile_pool(name="persist", bufs=1) as ppool,
            tc.tile_pool(name="stream", bufs=3) as spool,
            tc.tile_pool(name="sel", bufs=6) as selpool,
            tc.tile_pool(name="gather", bufs=4) as gpool,
            tc.tile_pool(name="psum", bufs=1, space="PSUM") as psum,
            tc.tile_pool(name="dram", bufs=1, space="DRAM") as dram,
        ):
            ident = cpool.tile([128, 128], F32)
            make_identity(nc, ident[:])
            wr1_s = cpool.tile([F, 64], F32, tag="wr1")
            wo1_s = cpool.tile([F, 64], F32, tag="wo1")
            w64 = {}
            for nm, d in [("wr2", wr2_d), ("wo2", wo2_d),
                          ("wr3", wr3_d), ("wo3", wo3_d)]:
                w64[nm] = cpool.tile([64, 64], F32, tag=nm, name=nm)
                nc.sync.dma_start(w64[nm][:], d[:])
            nc.sync.dma_start(wr1_s[:], wr1_d[:])
            nc.sync.dma_start(wo1_s[:], wo1_d[:])
            wfc_s = cpool.tile([64, 10], F32, tag="wfc")
            nc.sync.dma_start(wfc_s[:], wfc_d[:])
            bfc_s = cpool.tile([1, 10], F32, tag="bfc")
            nc.sync.dma_start(bfc_s[:], bfc_d[:])
            mask_s = cpool.tile([64, C3 * 128], F32, tag="mask")
            nc.sync.dma_start(mask_s[:], mask_d[:])
            idx_s = cpool.tile([128, p.WTOT], I16, tag="idx")
            nc.sync.dma_start(idx_s[:], idx_d[:])
            dst_s = cpool.tile([128, p.total_blocks], BF16, tag="dst")
            nc.sync.dma_start(dst_s[:], dst_d[:])
            xTall = cpool.tile([F, NLOC], F32, tag="xTall")
            nc.sync.dma_start(xTall[:], xT_d[:])
            zrow = cpool.tile([1, 128], BF16, tag="zrow")
            nc.vector.memset(zrow[:], 0.0)
            iota_s = cpool.tile([128, 128], BF16, tag="iota")
            nc.sync.dma_start(iota_s[:], iota_d[:])

            hT = [ppool.tile([64, NLOC], F32, tag="h1T", name="h1T"),
                  ppool.tile([64, C2 * 128], F32, tag="h2T", name="h2T"),
                  ppool.tile([64, C3 * 128], F32, tag="h3T", name="h3T")]
            rT = [ppool.tile([64, NLOC], F32, tag="r1T", name="r1T"),
                  ppool.tile([64, C2 * 128], F32, tag="r2T", name="r2T"),
                  ppool.tile([64, C3 * 128], F32, tag="r3T", name="r3T")]

            def store_z_chunks(li, zsrcT, c):
                """z chunk c -> bf16 node-major rows of the pair table."""
                wrel = wr1_s if li == 0 else w64["wr%d" % (li + 1)]
                sl = slice(c * 128, (c + 1) * 128)
                znT_p = psum.tile([64, 128], F32, tag="znT", bufs=2)
                nc.tensor.matmul(znT_p[:], lhsT=wrel[:], rhs=zsrcT[:, sl],
                                 start=True, stop=True)
                znT_s = spool.tile([64, 128], F32, tag="znT_s")
                nc.scalar.activation(znT_s[:], znT_p[:], ACTF.Copy)
                zn_p = psum.tile([128, 64], F32, tag="zn", bufs=2)
                nc.tensor.transpose(zn_p[:], znT_s[:], ident[:64, :64])
                zn_s = spool.tile([128, 64], BF16, tag="zn_s")
                nc.scalar.activation(zn_s[:], zn_p[:], ACTF.Copy)
                nc.sync.dma_start(z_own[li][c * 64:(c + 1) * 64, :], zn_s[:])

            def kick_ag(li):
                zp = p.Z[li] // 2
                nc.gpsimd.dma_start(z_own[li][zp - 1:zp, :], zrow[:])
                if not skip_collectives:
                    nc.gpsimd.collective_compute(
                        "AllGather", AX.bypass, replica_groups=rg,
                        ins=[z_own[li].opt()], outs=[z_tab[li].opt()])

            def root_precompute(li, srcT):
                wroot = wo1_s if li == 0 else w64["wo%d" % (li + 1)]
                for c in range(CL[li]):
                    sl = slice(c * 128, (c + 1) * 128)
                    rp = psum.tile([64, 128], F32, tag="znT", bufs=2)
                    nc.tensor.matmul(rp[:], lhsT=wroot[:], rhs=srcT[:, sl],
                                     start=True, stop=True)
                    nc.scalar.activation(rT[li][:, sl], rp[:], ACTF.Copy)

            for _rep in range(repeat):
                # pair tables: [node_rows/2, 128] bf16
                z_own = [dram.tile([p.Z[i] // 2, 128], BF16,
                                   name="z%do_%d" % (i, _rep))
                         for i in range(3)]
                z_tab = [dram.tile([NCORES * p.Z[i] // 2, 128], BF16,
                                   addr_space="Shared",
                                   name="z%dt_%d" % (i, _rep))
                         for i in range(3)]
                pool_in = dram.tile([64, 1], F32, name="pool_in_%d" % _rep)
                pool_out = dram.tile([64, 1], F32, addr_space="Shared",
                                     name="pool_out_%d" % _rep)
                for c in range(C1):
                    store_z_chunks(0, xTall, c)
                kick_ag(0)
                root_precompute(0, xTall)

                for li in range(3):
                    L = p.layers[li]
                    seen = set()
                    for (blk_start, nrows, chunks) in L["groups"]:
                        g = gpool.tile([128, nrows, 128], BF16, tag="G",
                                       name="G")
                        nc.gpsimd.dma_gather(
                            g[:], z_tab[li][:],
                            idx_s[:, blk_start * 8:(blk_start + nrows) * 8],
                            nrows * 128, nrows * 128, 128,
                            single_packet=False)
                        for (c, mms) in chunks:
                            seen.add(c)
                            ntot = len(mms)
                            if ntot == 0:
                                sl = slice(c * 128, (c + 1) * 128)
                                nc.vector.tensor_scalar_max(
                                    hT[li][:, sl], rT[li][:, sl], 0.0)
                                if li < 2 and c * 128 < p.Z[li + 1]:
                                    store_z_chunks(li + 1, hT[li], c)
                                continue
                            agg_p = psum.tile([64, 128], F32, tag="agg",
                                              bufs=3,
                                              name="agg%d_%d" % (li, c))
                            # batched sel builds over consecutive blocks
                            i = 0
                            b0 = 0
                            while b0 < ntot:
                                bn = min(4, ntot - b0)
                                gb = blk_start + mms[b0][0]
                                sel = selpool.tile([128, bn, 128], BF16,
                                                   tag="sel", name="sel")
                                nc.vector.tensor_tensor(
                                    out=sel[:],
                                    in0=dst_s[:, gb:gb + bn].unsqueeze(2)
                                    .to_broadcast([128, bn, 128]),
                                    in1=iota_s[:].unsqueeze(1)
                                    .to_broadcast([128, bn, 128]),
                                    op=AX.is_equal)
                                for b in range(bn):
                                    row, par = mms[b0 + b]
                                    nc.tensor.matmul(
                                        agg_p[:],
                                        lhsT=g[:, row,
                                               par * 64:(par + 1) * 64],
                                        rhs=sel[:, b, :],
                                        start=(i == 0),
                                        stop=(i == ntot - 1))
                                    i += 1
                                b0 += bn
                            sl = slice(c * 128, (c + 1) * 128)
                            nc.vector.tensor_tensor(
                                out=hT[li][:, sl], in0=agg_p[:],
                                in1=rT[li][:, sl], op=AX.add)
                            nc.vector.tensor_scalar_max(
                                hT[li][:, sl], hT[li][:, sl], 0.0)
                            if li < 2 and c * 128 < p.Z[li + 1]:
                                store_z_chunks(li + 1, hT[li], c)
                    for c in range(L["Cl"]):
                        if c not in seen:
                            sl = slice(c * 128, (c + 1) * 128)
                            nc.vector.tensor_scalar_max(
                                hT[li][:, sl], rT[li][:, sl], 0.0)
                            if li < 2 and c * 128 < p.Z[li + 1]:
                                store_z_chunks(li + 1, hT[li], c)
                    if li < 2:
                        kick_ag(li + 1)
                        root_precompute(li + 1, hT[li])

                hm = spool.tile([64, C3 * 128], F32, tag="hm")
                _tail(nc, tc, spool, psum, hm, hT, mask_s, wfc_s,
                      bfc_s, pool_in, pool_out, out_d, p, rg,
                      skip_collectives)

    nc.compile()
    return nc


def _tail(nc, tc, spool, psum, hm, hT, mask_s, wfc_s, bfc_s, pool_in,
          pool_out, out_d, p, rg, skip_collectives=False):
    nc.vector.tensor_tensor(out=hm[:], in0=hT[2][:],
                            in1=mask_s[:], op=AX.mult)
    psum_pool = spool.tile([64, 1], F32, tag="ppart")
    nc.vector.tensor_reduce(psum_pool[:], hm[:],
                            axis=mybir.AxisListType.X, op=AX.add)
    nc.sync.dma_start(pool_in[:], psum_pool[:])
    if not skip_collectives:
        nc.gpsimd.collective_compute(
            "AllReduce", AX.add, replica_groups=rg,
            ins=[pool_in.opt()], outs=[pool_out.opt()])
    pooled = spool.tile([64, 1], F32, tag="pooled")
    nc.sync.dma_start(pooled[:], pool_out[:])
    mean_s = spool.tile([64, 1], F32, tag="mean")
    nc.vector.tensor_scalar_mul(mean_s[:], pooled[:], 1.0 / max(p.n0, 1))
    lg_p = psum.tile([1, 10], F32, tag="lg")
    nc.tensor.matmul(lg_p[:], lhsT=mean_s[:], rhs=wfc_s[:],
                     start=True, stop=True)
    logits = spool.tile([1, 10], F32, tag="logits")
    nc.vector.tensor_tensor(out=logits[:], in0=lg_p[:],
                            in1=bfc_s[:], op=AX.add)
    mx = spool.tile([1, 1], F32, tag="mx")
    nc.vector.tensor_reduce(mx[:], logits[:],
                            axis=mybir.AxisListType.X, op=AX.max)
    nmx = spool.tile([1, 1], F32, tag="nmx")
    nc.vector.tensor_scalar_mul(nmx[:], mx[:], -1.0)
    es = spool.tile([1, 10], F32, tag="es")
    nc.scalar.activation(es[:], logits[:], ACTF.Exp, bias=nmx[:, 0:1])
    ssum = spool.tile([1, 1], F32, tag="ssum")
    nc.vector.tensor_reduce(ssum[:], es[:],
                            axis=mybir.AxisListType.X, op=AX.add)
    inv = spool.tile([1, 1], F32, tag="inv")
    nc.vector.reciprocal(inv[:], ssum[:])
    probs_s = spool.tile([1, 10], F32, tag="probs")
    nc.vector.tensor_scalar_mul(probs_s[:], es[:], inv[:, 0:1])
    nc.sync.dma_start(out_d[:], probs_s[:])


# ----------------------------------------------------------------------------
# Entry point
# ----------------------------------------------------------------------------

def _prep(inputs):
    x = np.ascontiguousarray(np.asarray(inputs["x"], np.float32))
    edge_index = np.asarray(inputs["edge_index"])
    batch = np.asarray(inputs["batch"])
    W = {k: np.ascontiguousarray(np.asarray(inputs[k], np.float32))
         for k in ["W_rel1", "W_root1", "W_rel2", "W_root2",
                   "W_rel3", "W_root3", "W_fc", "b_fc"]}
    p = build_plan(x, edge_index, batch)
    nc = build_program(p, W)
    return nc, _in_maps(p, W)


def _in_maps(p, W):
    in_maps = []
    for k in range(NCORES):
        in_maps.append({
            "xT": p.xT[k], "idx": p.idx[k], "mask": p.mask[k],
            "dstcol": p.dstcol[k],
            "W_rel1": W["W_rel1"], "W_root1": W["W_root1"],
            "W_rel2": W["W_rel2"], "W_root2": W["W_root2"],
            "W_rel3": W["W_rel3"], "W_root3": W["W_root3"],
            "W_fc": W["W_fc"], "b_fc": W["b_fc"].reshape(1, 10),
            "iota": np.tile(np.arange(128, dtype=np.float32)
                            .astype(ml_dtypes.bfloat16), (128, 1)),
        })
    return in_maps


def kernel(**inputs) -> np.ndarray:
    nc, in_maps = _prep(inputs)
    res = run_bass_kernel_spmd(nc, in_maps, list(range(NCORES)))
    return np.asarray(res.results[0]["probs"]).reshape(10).astype(np.float32)


# revision 6
# speedup vs baseline: 1.1531x; 1.1053x over previous
"""GNN message-passing (3x GraphConv + mean-pool + FC + softmax, graph 0 only)
on 8 Trainium2 NeuronCores — one-hot scatter + bf16 pair-table variant.

Strategy
--------
Nodes are partitioned interleaved across the 8 cores. ``segment_sum(h[src]) @
W_rel`` is computed as ``segment_sum(z[src])`` with ``z = h @ W_rel``; z
tables are stored in **bf16** as a "pair table" of 256-byte rows holding two
consecutive node vectors, which halves the AllGather traffic and keeps the
dma_gather element at the 256B minimum. Each layer:

  1. every core computes ``z`` for its own nodes (cast bf16); shards are
     AllGather-ed into a replicated DRAM pair table,
  2. each core dma_gathers the pair-rows of its in-edges in dense 128-edge
     blocks (~1 gather index per edge; per destination chunk the edges are
     split into even-parity and odd-parity block runs so each block reads one
     64-wide half of the gathered columns),
  3. each block is scattered into its destination columns with a PE matmul
     (bf16) against a one-hot selector built on the Vector engine (batched 4
     blocks per instruction), accumulated in PSUM per 128-destination chunk,
  4. the root term is added and ReLU-ed (feature-major), and the next
     layer's z chunk is written back node-major.

Only ``probs[0]`` is returned by the reference, so layers 2/3 are pruned to
the 1-hop/2-hop in-neighborhoods of graph-0 nodes (exact). Mean-pool partial
sums are AllReduce-d; every core redundantly computes the final FC + softmax.
"""

import numpy as np
import ml_dtypes

import concourse.bacc as bacc
import concourse.bass as bass
import concourse.mybir as mybir
import concourse.tile as tile
from concourse._compat import cdiv
from concourse.bass_utils import run_bass_kernel_spmd
from concourse.masks import make_identity

NCORES = 8
F32 = mybir.dt.float32
BF16 = mybir.dt.bfloat16
I16 = mybir.dt.int16
AX = mybir.AluOpType
ACTF = mybir.ActivationFunctionType

GROUP_ROWS = 64   # gather rows (of 128 idx) per dma_gather call


class Plan:
    pass


def build_plan(x, edge_index, batch):
    p = Plan()
    N, F = x.shape
    src = np.asarray(edge_index[0], dtype=np.int64)
    dst = np.asarray(edge_index[1], dtype=np.int64)
    batch = np.asarray(batch, dtype=np.int64)

    NPC = cdiv(N, NCORES)
    NLOC = cdiv(NPC + 1, 128) * 128
    p.N, p.F, p.NPC, p.NLOC = N, F, NPC, NLOC

    # --- pruning sets -------------------------------------------------------
    in_T0 = batch == 0
    p.n0 = int(in_T0.sum())
    e3 = in_T0[dst]
    in_T2 = in_T0.copy()
    in_T2[src[e3]] = True
    e2 = in_T2[dst]

    # --- balanced interleaved node->core assignment -------------------------
    nodes = np.arange(N)
    g0 = nodes[in_T0]
    g1 = nodes[in_T2 & ~in_T0]
    g2 = nodes[~in_T2]
    order = np.concatenate([g0, g1, g2])
    j = np.arange(N)
    node_core = np.empty(N, np.int64)
    pos = np.empty(N, np.int64)
    node_core[order] = j % NCORES
    pos[order] = j // NCORES
    n0_k = np.bincount(j[:len(g0)] % NCORES, minlength=NCORES)
    n2_k = np.bincount(j[:len(g0) + len(g1)] % NCORES, minlength=NCORES)
    core_of = node_core[dst]
    p.pos = pos
    p.node_core = node_core

    # --- chunk counts -------------------------------------------------------
    C1 = NLOC // 128
    C3 = max(1, cdiv(int(n0_k.max()), 128))
    C2 = max(C3, cdiv(int(n2_k.max()) + 1, 128))
    C2 = min(C2, C1)
    p.C = [C1, C2, C3]
    p.Z = [NLOC, NLOC, C2 * 128]          # node rows per core per table
    assert all((z * NCORES) // 2 < 32768 for z in p.Z)

    # --- per-layer block layout --------------------------------------------
    # Edges keyed by (dst chunk, parity of table row); per (chunk, parity)
    # the block count is the max over cores. The idx stream per chunk is
    # [even blocks][odd blocks]; groups own whole chunks.
    layers = []
    for li, (emask, Cl) in enumerate([(None, C1), (e2, C2), (e3, C3)]):
        es = src if emask is None else src[emask]
        ed = dst if emask is None else dst[emask]
        ecore = core_of if emask is None else core_of[emask]
        Zl = p.Z[li]
        if li < 2:
            srcpos = node_core[es] * NLOC + pos[es]
        else:
            srcpos = node_core[es] * Zl + pos[es]
        col = pos[ed]
        chunk = col // 128
        lane = col % 128
        par = srcpos % 2
        nblk = np.zeros((Cl, 2), np.int64)
        for k in range(NCORES):
            m = ecore == k
            for h in (0, 1):
                cnt = np.bincount(chunk[m & (par == h)], minlength=Cl)
                nblk[:, h] = np.maximum(nblk[:, h], cdiv(cnt, 128))
        layers.append(dict(li=li, Cl=Cl, Zl=Zl, srcpos=srcpos, col=col,
                           ecore=ecore, chunk=chunk, lane=lane, par=par,
                           nblk=nblk))
    p.layers = layers

    # --- global block/group layout (shared) ---------------------------------
    blk_off = 0
    for L in layers:
        L["blk_even"] = {}
        L["blk_odd"] = {}
        groups = []
        cur, cur_rows = [], 0
        for c in range(L["Cl"]):
            r = int(L["nblk"][c].sum())
            if cur and cur_rows + r > GROUP_ROWS:
                groups.append(cur)
                cur, cur_rows = [], 0
            cur.append(c)
            cur_rows += r
        if cur:
            groups.append(cur)
        # (blk_start, nrows, [(c, [(local_row, parity)] )])
        L["groups"] = []
        for grp in groups:
            blk_start = blk_off
            chunks = []
            for c in grp:
                mms = []
                for h, offmap in ((0, L["blk_even"]), (1, L["blk_odd"])):
                    nb = int(L["nblk"][c, h])
                    offmap[c] = blk_off
                    for b in range(nb):
                        mms.append((blk_off - blk_start + b, h))
                    blk_off += nb
                chunks.append((c, mms))
            nrows = blk_off - blk_start
            if nrows:
                L["groups"].append((blk_start, nrows, chunks))
    p.total_blocks = blk_off
    p.WTOT = blk_off * 8

    # --- per-core idx + dstcol arrays ---------------------------------------
    pad_pair = [(NLOC - 2) // 2, (NLOC - 2) // 2, (p.Z[2] - 2) // 2]
    p.idx = []
    p.dstcol = []
    for k in range(NCORES):
        flat = np.zeros(p.total_blocks * 128, np.int16)
        dcol = np.full(p.total_blocks * 128, 999.0, np.float32)
        for L in layers:
            li = L["li"]
            m = L["ecore"] == k
            sp_ = L["srcpos"][m]
            ch_ = L["chunk"][m]
            ln_ = L["lane"][m]
            pr_ = L["par"][m]
            for h, offmap in ((0, L["blk_even"]), (1, L["blk_odd"])):
                for c, off in offmap.items():
                    nb = int(L["nblk"][c, h])
                    if nb:
                        flat[off * 128:(off + nb) * 128] = pad_pair[li]
            order2 = np.lexsort((sp_, pr_, ch_))
            sc = ch_[order2]
            sh = pr_[order2]
            spv = sp_[order2]
            lnv = ln_[order2]
            key = sh.astype(np.int64) * (1 << 32) + sc
            newgrp = np.r_[True, np.diff(key) != 0]
            starts = np.flatnonzero(newgrp)
            lens = np.diff(np.r_[starts, len(key)])
            rank = np.arange(len(key)) - np.repeat(starts, lens)
            base_e = np.array([L["blk_even"].get(c, 0)
                               for c in range(L["Cl"])])
            base_o = np.array([L["blk_odd"].get(c, 0)
                               for c in range(L["Cl"])])
            base = np.where(sh == 0, base_e[sc], base_o[sc])
            slot = base * 128 + rank
            flat[slot] = (spv // 2).astype(np.int16)
            dcol[slot] = lnv.astype(np.float32)
        wrapped = flat.reshape(-1, 16).T.copy()
        p.idx.append(np.tile(wrapped, (8, 1)))
        p.dstcol.append(np.ascontiguousarray(
            dcol.reshape(p.total_blocks, 128).T.astype(ml_dtypes.bfloat16)))

    # --- per-core xT --------------------------------------------------------
    p.xT = []
    for k in range(NCORES):
        xp = np.zeros((NLOC, F), np.float32)
        kn = nodes[node_core == k]
        xp[pos[kn]] = x[kn]
        p.xT.append(np.ascontiguousarray(xp.T))

    # --- per-core pool mask -------------------------------------------------
    p.mask = []
    for k in range(NCORES):
        msk = np.zeros(C3 * 128, np.float32)
        msk[:n0_k[k]] = 1.0
        p.mask.append(np.broadcast_to(msk, (64, C3 * 128)).copy())

    return p


# ----------------------------------------------------------------------------
# Device program
# ----------------------------------------------------------------------------

def build_program(p, W, skip_collectives=False, repeat=1):
    nc = bacc.Bacc("TRN2")
    NLOC, F = p.NLOC, p.F
    C1, C2, C3 = p.C
    CL = [C1, C2, C3]

    xT_d = nc.dram_tensor("xT", [F, NLOC], F32, kind="ExternalInput")
    idx_d = nc.dram_tensor("idx", [128, p.WTOT], I16, kind="ExternalInput")
    dst_d = nc.dram_tensor("dstcol", [128, p.total_blocks], BF16,
                           kind="ExternalInput")
    mask_d = nc.dram_tensor("mask", [64, C3 * 128], F32, kind="ExternalInput")
    wr1_d = nc.dram_tensor("W_rel1", [F, 64], F32, kind="ExternalInput")
    wo1_d = nc.dram_tensor("W_root1", [F, 64], F32, kind="ExternalInput")
    wr2_d = nc.dram_tensor("W_rel2", [64, 64], F32, kind="ExternalInput")
    wo2_d = nc.dram_tensor("W_root2", [64, 64], F32, kind="ExternalInput")
    wr3_d = nc.dram_tensor("W_rel3", [64, 64], F32, kind="ExternalInput")
    wo3_d = nc.dram_tensor("W_root3", [64, 64], F32, kind="ExternalInput")
    wfc_d = nc.dram_tensor("W_fc", [64, 10], F32, kind="ExternalInput")
    bfc_d = nc.dram_tensor("b_fc", [1, 10], F32, kind="ExternalInput")
    iota_d = nc.dram_tensor("iota", [128, 128], BF16, kind="ExternalInput")
    out_d = nc.dram_tensor("probs", [1, 10], F32, kind="ExternalOutput")

    rg = [list(range(NCORES))]

    with tile.TileContext(nc) as tc:
        with (
            tc.tile_pool(name="const", bufs=1) as cpool,
            tc.tile_pool(name="persist", bufs=1) as ppool,
            tc.tile_pool(name="stream", bufs=3) as spool,
            tc.tile_pool(name="sel", bufs=6) as selpool,
            tc.tile_pool(name="gather", bufs=4) as gpool,
            tc.tile_pool(name="psum", bufs=1, space="PSUM") as psum,
            tc.tile_pool(name="dram", bufs=1, space="DRAM") as dram,
        ):
            ident = cpool.tile([128, 128], F32)
            make_identity(nc, ident[:])
            wr1_s = cpool.tile([F, 64], F32, tag="wr1")
            wo1_s = cpool.tile([F, 64], F32, tag="wo1")
            w64 = {}
            for nm, d in [("wr2", wr2_d), ("wo2", wo2_d),
                          ("wr3", wr3_d), ("wo3", wo3_d)]:
                w64[nm] = cpool.tile([64, 64], F32, tag=nm, name=nm)
                nc.sync.dma_start(w64[nm][:], d[:])
            nc.sync.dma_start(wr1_s[:], wr1_d[:])
            nc.sync.dma_start(wo1_s[:], wo1_d[:])
            wfc_s = cpool.tile([64, 10], F32, tag="wfc")
            nc.sync.dma_start(wfc_s[:], wfc_d[:])
            bfc_s = cpool.tile([1, 10], F32, tag="bfc")
            nc.sync.dma_start(bfc_s[:], bfc_d[:])
            mask_s = cpool.tile([64, C3 * 128], F32, tag="mask")
            nc.sync.dma_start(mask_s[:], mask_d[:])
            idx_s = cpool.tile([128, p.WTOT], I16, tag="idx")
            nc.sync.dma_start(idx_s[:], idx_d[:])
            dst_s = cpool.tile([128, p.total_blocks], BF16, tag="dst")
            nc.sync.dma_start(dst_s[:], dst_d[:])
            xTall = cpool.tile([F, NLOC], F32, tag="xTall")
            nc.sync.dma_start(xTall[:], xT_d[:])
            zrow = cpool.tile([1, 128], BF16, tag="zrow")
            nc.vector.memset(zrow[:], 0.0)
            iota_s = cpool.tile([128, 128], BF16, tag="iota")
            nc.sync.dma_start(iota_s[:], iota_d[:])

            hT = [ppool.tile([64, NLOC], F32, tag="h1T", name="h1T"),
                  ppool.tile([64, C2 * 128], F32, tag="h2T", name="h2T"),
                  ppool.tile([64, C3 * 128], F32, tag="h3T", name="h3T")]
            rT = [ppool.tile([64, NLOC], F32, tag="r1T", name="r1T"),
                  ppool.tile([64, C2 * 128], F32, tag="r2T", name="r2T"),
                  ppool.tile([64, C3 * 128], F32, tag="r3T", name="r3T")]

            def store_z_chunks(li, zsrcT, c):
                """z chunk c -> bf16 node-major rows of the pair table."""
                wrel = wr1_s if li == 0 else w64["wr%d" % (li + 1)]
                sl = slice(c * 128, (c + 1) * 128)
                znT_p = psum.tile([64, 128], F32, tag="znT", bufs=2)
                nc.tensor.matmul(znT_p[:], lhsT=wrel[:], rhs=zsrcT[:, sl],
                                 start=True, stop=True)
                znT_s = spool.tile([64, 128], F32, tag="znT_s")
                nc.scalar.activation(znT_s[:], znT_p[:], ACTF.Copy)
                zn_p = psum.tile([128, 64], F32, tag="zn", bufs=2)
                nc.tensor.transpose(zn_p[:], znT_s[:], ident[:64, :64])
                zn_s = spool.tile([128, 64], BF16, tag="zn_s")
                nc.scalar.activation(zn_s[:], zn_p[:], ACTF.Copy)
                nc.sync.dma_start(z_own[li][c * 64:(c + 1) * 64, :], zn_s[:])

            def kick_ag(li):
                zp = p.Z[li] // 2
                nc.gpsimd.dma_start(z_own[li][zp - 1:zp, :], zrow[:])
                if not skip_collectives:
                    nc.gpsimd.collective_compute(
                        "AllGather", AX.bypass, replica_groups=rg,
                        ins=[z_own[li].opt()], outs=[z_tab[li].opt()])

            def root_precompute(li, srcT):
                wroot = wo1_s if li == 0 else w64["wo%d" % (li + 1)]
                for c in range(CL[li]):
                    sl = slice(c * 128, (c + 1) * 128)
                    rp = psum.tile([64, 128], F32, tag="znT", bufs=2)
                    nc.tensor.matmul(rp[:], lhsT=wroot[:], rhs=srcT[:, sl],
                                     start=True, stop=True)
                    nc.scalar.activation(rT[li][:, sl], rp[:], ACTF.Copy)

            for _rep in range(repeat):
                # pair tables: [node_rows/2, 128] bf16
                z_own = [dram.tile([p.Z[i] // 2, 128], BF16,
                                   name="z%do_%d" % (i, _rep))
                         for i in range(3)]
                z_tab = [dram.tile([NCORES * p.Z[i] // 2, 128], BF16,
                                   addr_space="Shared",
                                   name="z%dt_%d" % (i, _rep))
                         for i in range(3)]
                pool_in = dram.tile([64, 1], F32, name="pool_in_%d" % _rep)
                pool_out = dram.tile([64, 1], F32, addr_space="Shared",
                                     name="pool_out_%d" % _rep)
                for c in range(C1):
                    store_z_chunks(0, xTall, c)
                kick_ag(0)
                root_precompute(0, xTall)

                for li in range(3):
                    L = p.layers[li]
                    seen = set()
                    for (blk_start, nrows, chunks) in L["groups"]:
                        g = gpool.tile([128, nrows, 128], BF16, tag="G",
                                       name="G")
                        nc.gpsimd.dma_gather(
                            g[:], z_tab[li][:],
                            idx_s[:, blk_start * 8:(blk_start + nrows) * 8],
                            nrows * 128, nrows * 128, 128,
                            single_packet=False)
                        for (c, mms) in chunks:
                            seen.add(c)
                            ntot = len(mms)
                            if ntot == 0:
                                sl = slice(c * 128, (c + 1) * 128)
                                nc.vector.tensor_scalar_max(
                                    hT[li][:, sl], rT[li][:, sl], 0.0)
                                if li < 2 and c * 128 < p.Z[li + 1]:
                                    store_z_chunks(li + 1, hT[li], c)
                                continue
                            agg_p = psum.tile([64, 128], F32, tag="agg",
                                              bufs=3,
                                              name="agg%d_%d" % (li, c))
                            # root term folded into the PSUM accumulation
                            # (identity matmul) so the finalize needs no DVE
                            nc.tensor.matmul(agg_p[:], lhsT=ident[:64, :64],
                                             rhs=rT[li][:, c * 128:
                                                        (c + 1) * 128],
                                             start=True, stop=False)
                            # batched sel builds over consecutive blocks
                            i = 1
                            b0 = 0
                            while b0 < ntot:
                                bn = min(4, ntot - b0)
                                gb = blk_start + mms[b0][0]
                                sel = selpool.tile([128, bn, 128], BF16,
                                                   tag="sel", name="sel")
                                nc.vector.tensor_tensor(
                                    out=sel[:],
                                    in0=dst_s[:, gb:gb + bn].unsqueeze(2)
                                    .to_broadcast([128, bn, 128]),
                                    in1=iota_s[:].unsqueeze(1)
                                    .to_broadcast([128, bn, 128]),
                                    op=AX.is_equal)
                                for b in range(bn):
                                    row, par = mms[b0 + b]
                                    nc.tensor.matmul(
                                        agg_p[:],
                                        lhsT=g[:, row,
                                               par * 64:(par + 1) * 64],
                                        rhs=sel[:, b, :],
                                        start=False,
                                        stop=(i == ntot))
                                    i += 1
                                b0 += bn
                            sl = slice(c * 128, (c + 1) * 128)
                            nc.scalar.activation(hT[li][:, sl], agg_p[:],
                                                 ACTF.Relu)
                            if li < 2 and c * 128 < p.Z[li + 1]:
                                store_z_chunks(li + 1, hT[li], c)
                    for c in range(L["Cl"]):
                        if c not in seen:
                            sl = slice(c * 128, (c + 1) * 128)
                            nc.vector.tensor_scalar_max(
                                hT[li][:, sl], rT[li][:, sl], 0.0)
                            if li < 2 and c * 128 < p.Z[li + 1]:
                                store_z_chunks(li + 1, hT[li], c)
                    if li < 2:
                        kick_ag(li + 1)
                        root_precompute(li + 1, hT[li])

                hm = spool.tile([64, C3 * 128], F32, tag="hm")
                _tail(nc, tc, spool, psum, hm, hT, mask_s, wfc_s,
                      bfc_s, pool_in, pool_out, out_d, p, rg,
                      skip_collectives)

    nc.compile()
    return nc


def _tail(nc, tc, spool, psum, hm, hT, mask_s, wfc_s, bfc_s, pool_in,
          pool_out, out_d, p, rg, skip_collectives=False):
    nc.vector.tensor_tensor(out=hm[:], in0=hT[2][:],
                            in1=mask_s[:], op=AX.mult)
    psum_pool = spool.tile([64, 1], F32, tag="ppart")
    nc.vector.tensor_reduce(psum_pool[:], hm[:],
                            axis=mybir.AxisListType.X, op=AX.add)
    nc.sync.dma_start(pool_in[:], psum_pool[:])
    if not skip_collectives:
        nc.gpsimd.collective_compute(
            "AllReduce", AX.add, replica_groups=rg,
            ins=[pool_in.opt()], outs=[pool_out.opt()])
    pooled = spool.tile([64, 1], F32, tag="pooled")
    nc.sync.dma_start(pooled[:], pool_out[:])
    mean_s = spool.tile([64, 1], F32, tag="mean")
    nc.vector.tensor_scalar_mul(mean_s[:], pooled[:], 1.0 / max(p.n0, 1))
    lg_p = psum.tile([1, 10], F32, tag="lg")
    nc.tensor.matmul(lg_p[:], lhsT=mean_s[:], rhs=wfc_s[:],
                     start=True, stop=True)
    logits = spool.tile([1, 10], F32, tag="logits")
    nc.vector.tensor_tensor(out=logits[:], in0=lg_p[:],
                            in1=bfc_s[:], op=AX.add)
    mx = spool.tile([1, 1], F32, tag="mx")
    nc.vector.tensor_reduce(mx[:], logits[:],
                            axis=mybir.AxisListType.X, op=AX.max)
    nmx = spool.tile([1, 1], F32, tag="nmx")
    nc.vector.tensor_scalar_mul(nmx[:], mx[:], -1.0)
    es = spool.tile([1, 10], F32, tag="es")
    nc.scalar.activation(es[:], logits[:], ACTF.Exp, bias=nmx[:, 0:1])
    ssum = spool.tile([1, 1], F32, tag="ssum")
    nc.vector.tensor_reduce(ssum[:], es[:],
                            axis=mybir.AxisListType.X, op=AX.add)
    inv = spool.tile([1, 1], F32, tag="inv")
    nc.vector.reciprocal(inv[:], ssum[:])
    probs_s = spool.tile([1, 10], F32, tag="probs")
    nc.vector.tensor_scalar_mul(probs_s[:], es[:], inv[:, 0:1])
    nc.sync.dma_start(out_d[:], probs_s[:])


# ----------------------------------------------------------------------------
# Entry point
# ----------------------------------------------------------------------------

def _prep(inputs):
    x = np.ascontiguousarray(np.asarray(inputs["x"], np.float32))
    edge_index = np.asarray(inputs["edge_index"])
    batch = np.asarray(inputs["batch"])
    W = {k: np.ascontiguousarray(np.asarray(inputs[k], np.float32))
         for k in ["W_rel1", "W_root1", "W_rel2", "W_root2",
                   "W_rel3", "W_root3", "W_fc", "b_fc"]}
    p = build_plan(x, edge_index, batch)
    nc = build_program(p, W)
    return nc, _in_maps(p, W)


def _in_maps(p, W):
    in_maps = []
    for k in range(NCORES):
        in_maps.append({
            "xT": p.xT[k], "idx": p.idx[k], "mask": p.mask[k],
            "dstcol": p.dstcol[k],
            "W_rel1": W["W_rel1"], "W_root1": W["W_root1"],
            "W_rel2": W["W_rel2"], "W_root2": W["W_root2"],
            "W_rel3": W["W_rel3"], "W_root3": W["W_root3"],
            "W_fc": W["W_fc"], "b_fc": W["b_fc"].reshape(1, 10),
            "iota": np.tile(np.arange(128, dtype=np.float32)
                            .astype(ml_dtypes.bfloat16), (128, 1)),
        })
    return in_maps


def kernel(**inputs) -> np.ndarray:
    nc, in_maps = _prep(inputs)
    res = run_bass_kernel_spmd(nc, in_maps, list(range(NCORES)))
    return np.asarray(res.results[0]["probs"]).reshape(10).astype(np.float32)
